# revision 6
# baseline (speedup 1.0000x reference)
"""Decorrelation loss kernel for Trainium2 (Bass/Tile), 8-core SPMD.

Problem: x [8, 1024, 64, 64] f32.
  xf = x.reshape(B, C, HW)
  dot = xf @ xf^T / 0.2            per batch  -> [C, C]
  loss = mean(-log(diag(softmax(dot, -1)) + 1e-10))

Sharding: data-parallel over batch B across the 8 cores; each core handles
one batch element and emits its 1024 per-row losses; host averages.

Per-core pipeline:
  1. SWDGE cast-DMA loads x_b [1024, 4096] f32 from HBM into SBUF as bf16
     (cast happens inside the DMA datapath; no compute engine involved).
  2. HWDGE DMA-transpose (xbar) builds xT in SBUF: [128p, 32q, 1024c],
     where n = 4096 contraction index maps to (p, q).  Any consistent
     bijection works since contraction is permutation-invariant.
  3. TensorE computes G = xf xf^T row-block by row-block: for each of 8
     row blocks i, two [128, 512] f32 PSUM tiles accumulate 32 bf16
     matmuls each (contraction over q).
  4. Row softmax stats: DVE reduce_max over PSUM -> row max m; ACT
     computes exp(5*g - 5*m) with accum_out giving the row sum S; the
     diagonal of G is extracted with an identity mask + tensor_tensor_reduce.
  5. loss_row = -Ln(diag_softmax + 1e-10) computed on ACT (Ln LUT), so the
     result matches the jax-on-neuron reference bit-for-bit.
"""

import numpy as np

import concourse.bass as bass
import concourse.mybir as mybir
from concourse import bacc
from concourse.tile import TileContext
from concourse.bass_utils import run_bass_kernel_spmd
from concourse.masks import make_identity

B, C, HW = 8, 1024, 4096
P = 128                     # partitions
NB = C // P                 # 8 row blocks
KQ = HW // P                # 32 contraction sub-blocks
NJ = C // 512               # 2 column tiles of 512
INV_T = 5.0                 # 1 / TEMPERATURE
EPS = 1e-10

F32 = mybir.dt.float32
BF16 = mybir.dt.bfloat16


def _build(debug=False):
    nc = bacc.Bacc(None, target_bir_lowering=False)

    x = nc.dram_tensor("x", [C, HW], F32, kind="ExternalInput")
    loss = nc.dram_tensor("loss", [P, NB], F32, kind="ExternalOutput")
    if debug:
        g_dbg = nc.dram_tensor("g_dbg", [P, C], F32, kind="ExternalOutput")
        s_dbg = nc.dram_tensor("s_dbg", [P, 4], F32, kind="ExternalOutput")

    with TileContext(nc) as tc:
        with (
            tc.tile_pool(name="singles", bufs=1) as singles,
            tc.tile_pool(name="xchunk", bufs=3) as xchunk_pool,
            tc.tile_pool(name="xt", bufs=1) as xt_pool,
            tc.tile_pool(name="psum", bufs=4, space="PSUM") as psum_pool,
            tc.tile_pool(name="escr", bufs=3) as escr_pool,
            tc.tile_pool(name="dscr", bufs=2) as dscr_pool,
            tc.tile_pool(name="stats", bufs=16) as stats_pool,
        ):
            ident = singles.tile([P, P], F32)
            make_identity(nc, ident)

            lossacc = singles.tile([P, NB], F32)

            # xT in SBUF, split into lo/hi column halves so matmuls on the
            # lo half can start while the hi half is still loading.
            # Layout: [128 p, 32 q, 512 c] bf16; n = p*KQ + q (any bijection ok).
            xt_lo = xt_pool.tile([P, KQ, 512], BF16, tag="xt_lo")
            xt_hi = xt_pool.tile([P, KQ, 512], BF16, tag="xt_hi")

            def xt_slice(c0, c1):
                """bf16 xT AP for columns [c0, c1) at fixed q (set by caller)."""
                assert (c0 < 512) == (c1 <= 512)
                if c1 <= 512:
                    return xt_lo, c0
                return xt_hi, c0 - 512

            # 1+2. load (cast) + transpose, chunk by chunk
            for cb in range(NB):
                xchunk = xchunk_pool.tile([P, HW], BF16, tag="xchunk")
                nc.gpsimd.dma_start(out=xchunk[:], in_=x[cb * P:(cb + 1) * P, :])
                dst, off = xt_slice(cb * P, (cb + 1) * P)
                nc.sync.dma_start_transpose(
                    out=dst[:, :, off:off + P], in_=xchunk[:]
                )

            # 3-5. per row block
            for i in range(NB):
                ps = []
                for j in range(NJ):
                    ps.append(
                        psum_pool.tile([P, 512], F32, tag="ps", name=f"ps_{i}_{j}")
                    )
                for q in range(KQ):
                    lhs_t, lhs_off = xt_slice(i * P, (i + 1) * P)
                    lhsT = lhs_t[:, q, lhs_off:lhs_off + P]
                    for j in range(NJ):
                        rhs_t, rhs_off = xt_slice(j * 512, (j + 1) * 512)
                        nc.tensor.matmul(
                            ps[j],
                            lhsT=lhsT,
                            rhs=rhs_t[:, q, rhs_off:rhs_off + 512],
                            start=(q == 0),
                            stop=(q == KQ - 1),
                        )

                # row max over the full 1024 row (raw g, unscaled)
                m0 = stats_pool.tile([P, 1], F32, tag="m0")
                m1 = stats_pool.tile([P, 1], F32, tag="m1")
                nc.vector.reduce_max(m0, ps[0][:], axis=mybir.AxisListType.X)
                nc.vector.reduce_max(m1, ps[1][:], axis=mybir.AxisListType.X)
                mm = stats_pool.tile([P, 1], F32, tag="mm")
                nc.vector.tensor_max(mm, m0, m1)

                # exp(5*(g - m)) with row-sum accumulation.  The subtract is a
                # separate DVE op so the diagonal (max) element cancels to an
                # exact 0.0 before the ACT exp (ACT's internal scale*in+bias
                # path does not cancel bit-exactly).
                ssum = []
                for j in range(NJ):
                    sub = escr_pool.tile([P, 512], F32, tag="sub", name=f"sub_{i}_{j}")
                    nc.vector.tensor_scalar(
                        out=sub[:],
                        in0=ps[j][:],
                        scalar1=mm[:],
                        scalar2=None,
                        op0=mybir.AluOpType.subtract,
                    )
                    e_scr = escr_pool.tile([P, 512], F32, tag="e", name=f"e_{i}_{j}")
                    s_j = stats_pool.tile([P, 1], F32, tag=f"s{j}", name=f"s_{i}_{j}")
                    nc.scalar.activation(
                        out=e_scr[:],
                        in_=sub[:],
                        func=mybir.ActivationFunctionType.Exp,
                        bias=0.0,
                        scale=INV_T,
                        accum_out=s_j[:],
                    )
                    ssum.append(s_j)
                S = stats_pool.tile([P, 1], F32, tag="S")
                nc.vector.tensor_add(S, ssum[0], ssum[1])

                # diagonal of G for this row block: block (i, i)
                jd, off = divmod(i * P, 512)
                dscr = dscr_pool.tile([P, P], F32, tag="dscr")
                nc.vector.tensor_mul(dscr[:], ps[jd][:, off:off + P], ident[:])
                dsum = stats_pool.tile([P, 1], F32, tag="dsum")
                nc.vector.reduce_sum(dsum, dscr[:], axis=mybir.AxisListType.X)

                # diag softmax = exp(5*(d - m)) / S
                dm = stats_pool.tile([P, 1], F32, tag="dm")
                nc.vector.tensor_sub(dm, dsum, mm)
                ec = stats_pool.tile([P, 1], F32, tag="ec")
                nc.scalar.activation(
                    out=ec[:],
                    in_=dm[:],
                    func=mybir.ActivationFunctionType.Exp,
                    bias=0.0,
                    scale=INV_T,
                )
                rS = stats_pool.tile([P, 1], F32, tag="rS")
                nc.vector.reciprocal(rS, S)
                r = stats_pool.tile([P, 1], F32, tag="r")
                nc.vector.tensor_mul(r, ec, rS)

                # v = diag + eps (separate f32 rounding step, like the ref)
                v = stats_pool.tile([P, 1], F32, tag="v")
                nc.vector.tensor_scalar_add(v, r, EPS)
                # loss = -Ln(v) on the ACT Ln LUT
                lnv = stats_pool.tile([P, 1], F32, tag="lnv")
                nc.scalar.activation(
                    out=lnv[:],
                    in_=v[:],
                    func=mybir.ActivationFunctionType.Ln,
                    bias=0.0,
                    scale=1.0,
                )
                nc.vector.tensor_scalar_mul(lossacc[:, i:i + 1], lnv, -1.0)

                if debug and i == 0:
                    gcopy = escr_pool.tile([P, 512], F32, tag="gcopy")
                    for j in range(NJ):
                        nc.vector.tensor_copy(gcopy[:], ps[j][:])
                        nc.sync.dma_start(
                            out=g_dbg[:, j * 512:(j + 1) * 512], in_=gcopy[:]
                        )
                    sdbg = stats_pool.tile([P, 4], F32, tag="sdbg")
                    nc.vector.tensor_copy(sdbg[:, 0:1], mm)
                    nc.vector.tensor_copy(sdbg[:, 1:2], S)
                    nc.vector.tensor_copy(sdbg[:, 2:3], dsum)
                    nc.vector.tensor_copy(sdbg[:, 3:4], r)
                    nc.sync.dma_start(out=s_dbg[:], in_=sdbg[:])

            nc.sync.dma_start(out=loss[:], in_=lossacc[:])

    nc.finalize()
    return nc


_CACHED = {}


def _get_nc(debug=False):
    key = bool(debug)
    if key not in _CACHED:
        _CACHED[key] = _build(debug=debug)
    return _CACHED[key]


def kernel(x, _debug=False):
    x = np.ascontiguousarray(np.asarray(x, dtype=np.float32)).reshape(B, C, HW)
    nc = _get_nc(debug=_debug)
    in_maps = [{"x": x[b]} for b in range(B)]
    res = run_bass_kernel_spmd(nc, in_maps, core_ids=list(range(B)))
    rows = np.stack([res.results[b]["loss"] for b in range(B)])  # [B, 128, NB]
    # loss[p, i] is row c = i*128 + p of batch b
    per_row = np.transpose(rows, (0, 2, 1)).reshape(B, C)
    out = np.float32(np.mean(per_row.astype(np.float32)))
    kernel._last = res
    kernel._per_row = per_row
    return np.asarray(out, dtype=np.float32)


# revision 7
# speedup vs baseline: 1.0278x; 1.0278x over previous
"""Decorrelation loss kernel for Trainium2 (Bass/Tile), 8-core SPMD.

Problem: x [8, 1024, 64, 64] f32.
  xf = x.reshape(B, C, HW)
  dot = xf @ xf^T / 0.2            per batch  -> [C, C]
  loss = mean(-log(diag(softmax(dot, -1)) + 1e-10))

Sharding: data-parallel over batch B across the 8 cores; each core handles
one batch element and emits per-row losses; the host averages.

Per-core pipeline:
  1. SWDGE cast-DMA loads x_b [1024, 4096] f32 from HBM into SBUF as bf16,
     one 128-row chunk at a time (cast happens inside the DMA datapath).
  2. HWDGE xbar DMA-transpose builds xT in SBUF: [128p, 32q, c] (n = q*128+p).
  3. TensorE accumulates G row-strips in PSUM (bf16 inputs, f32 accum).
     Only the lower-triangle wedge of G (strip i: cols [0, 128*(i+1))) is
     computed: for N(0,1) inputs the Gram diagonal ||x_c||^2/T ~ 20480
     exceeds off-diagonals (~+-1500) by >> 21, so every softmax term of the
     skipped upper triangle underflows to +0.0 in f32 and the diagonal is
     always the row max -- the wedge row-stats are bit-identical to the
     full-row stats.  This is validated per run: the kernel also outputs
     per-row max and wedge off-diagonal margin, and the host falls back to
     a full dense computation if the margin guarantee ever fails.
  4. Row stats on DVE/ACT: row max m (wedge), g-m (exact 0 at the max),
     ACT exp with accum_out -> row sum S; diagonal via identity mask.
  5. Final -log(diag_softmax + 1e-10) on the ACT Ln LUT (bit-matches the
     jax-on-neuron reference), deferred to a single pass at the end so the
     ACT table is not reloaded per strip.
"""

import numpy as np

import concourse.bass as bass
import concourse.mybir as mybir
from concourse import bacc
from concourse.tile import TileContext
from concourse.bass_utils import run_bass_kernel_spmd
from concourse.masks import make_identity

B, C, HW = 8, 1024, 4096
P = 128                     # partitions
NB = C // P                 # 8 row strips
KQ = HW // P                # 32 contraction sub-blocks
INV_T = 5.0                 # 1 / TEMPERATURE
EPS = 1e-10
NEG_BIG = -1.0e30

F32 = mybir.dt.float32
BF16 = mybir.dt.bfloat16
AF = mybir.ActivationFunctionType
AL = mybir.AluOpType
AX = mybir.AxisListType

# Host-side validity condition for the wedge shortcut (see module docstring):
# every skipped exp argument is 5*(g_cd - m_d) <= 5*(spread - min_margin);
# f32 exp underflows to +0.0 below exp(-103.97), i.e. argument < -104.
# Require margin - spread > 30 (raw units; *5 = 150 << -104 threshold met
# with slack), and the diagonal strictly dominating every wedge off-diag.
MARGIN_SLACK = 30.0


def _build(debug=False):
    nc = bacc.Bacc(None, target_bir_lowering=False)

    x = nc.dram_tensor("x", [C, HW], F32, kind="ExternalInput")
    loss = nc.dram_tensor("loss", [P, NB], F32, kind="ExternalOutput")
    mrow = nc.dram_tensor("mrow", [P, NB], F32, kind="ExternalOutput")
    margin = nc.dram_tensor("margin", [P, NB], F32, kind="ExternalOutput")

    with TileContext(nc) as tc:
        with (
            tc.tile_pool(name="singles", bufs=1) as singles,
            tc.tile_pool(name="xchunk", bufs=3) as xchunk_pool,
            tc.tile_pool(name="xt", bufs=1) as xt_pool,
            tc.tile_pool(name="psum", bufs=8, space="PSUM") as psum_pool,
            tc.tile_pool(name="escr", bufs=3) as escr_pool,
            tc.tile_pool(name="dscr", bufs=2) as dscr_pool,
            tc.tile_pool(name="stats", bufs=16) as stats_pool,
        ):
            ident = singles.tile([P, P], F32)
            make_identity(nc, ident)

            # per-strip stashes, consumed by the final pass
            Sacc = singles.tile([P, NB], F32)
            ecacc = singles.tile([P, NB], F32)
            macc = singles.tile([P, NB], F32)
            offacc = singles.tile([P, NB], F32)

            # xT in SBUF, split into lo/hi column halves: [128p, 32q, 512c]
            # bf16, n = q*128 + p.  Split so wedge matmuls that only touch
            # one half don't dep-wait on the other half's transposes.
            xt_lo = xt_pool.tile([P, KQ, 512], BF16, tag="xt_lo")
            xt_hi = xt_pool.tile([P, KQ, 512], BF16, tag="xt_hi")

            def xt_half(c0):
                return (xt_lo, c0) if c0 < 512 else (xt_hi, c0 - 512)

            # 1+2. cast-load + transpose, chunk by chunk (natural order:
            # lower-wedge strips need low chunks first)
            for cb in range(NB):
                xchunk = xchunk_pool.tile([P, HW], BF16, tag="xchunk",
                                          name=f"xchunk_{cb}")
                nc.gpsimd.dma_start(out=xchunk[:], in_=x[cb * P:(cb + 1) * P, :])
                dst, off = xt_half(cb * P)
                nc.sync.dma_start_transpose(
                    out=dst[:, :, off:off + P], in_=xchunk[:]
                )

            # 3+4. per row strip: wedge = cols [0, 128*(i+1))
            for i in range(NB):
                wedge_end = P * (i + 1)
                # column pieces, split at the 512 boundary
                spans = []
                if wedge_end <= 512:
                    spans.append((0, wedge_end))
                else:
                    spans.append((0, 512))
                    spans.append((512, wedge_end))

                ps = []
                for (c0, c1) in spans:
                    ps.append(psum_pool.tile(
                        [P, 512], F32, tag="ps", name=f"ps_{i}_{c0}"))
                lhs_t, lhs_off = xt_half(i * P)
                for q in range(KQ):
                    lhsT = lhs_t[:, q, lhs_off:lhs_off + P]
                    for (c0, c1), pst in zip(spans, ps):
                        rhs_t, ro = xt_half(c0)
                        nc.tensor.matmul(
                            pst[:, 0:c1 - c0],
                            lhsT=lhsT,
                            rhs=rhs_t[:, q, ro:ro + (c1 - c0)],
                            start=(q == 0),
                            stop=(q == KQ - 1),
                        )

                # row max over the wedge (raw g, unscaled)
                mparts = []
                for k, ((c0, c1), pst) in enumerate(zip(spans, ps)):
                    mk = stats_pool.tile([P, 1], F32, tag=f"m{k}",
                                         name=f"m_{i}_{k}")
                    nc.vector.reduce_max(mk, pst[:, 0:c1 - c0], axis=AX.X)
                    mparts.append(mk)
                if len(mparts) == 1:
                    mm = mparts[0]
                else:
                    mm = stats_pool.tile([P, 1], F32, tag="mm", name=f"mm_{i}")
                    nc.vector.tensor_max(mm, mparts[0], mparts[1])
                nc.vector.tensor_copy(macc[:, i:i + 1], mm)

                # exp(5*(g - m)) with row-sum accumulation; subtract is a
                # separate DVE op so the max element cancels to exact 0.0
                subs = []
                for k, ((c0, c1), pst) in enumerate(zip(spans, ps)):
                    w = c1 - c0
                    sub = escr_pool.tile([P, 512], F32, tag="sub",
                                         name=f"sub_{i}_{k}")
                    nc.vector.tensor_scalar(
                        out=sub[:, 0:w], in0=pst[:, 0:w],
                        scalar1=mm[:], scalar2=None, op0=AL.subtract,
                    )
                    subs.append(sub)
                    e_scr = escr_pool.tile([P, 512], F32, tag="e",
                                           name=f"e_{i}_{k}")
                    sk = stats_pool.tile([P, 1], F32, tag=f"s{k}",
                                         name=f"s_{i}_{k}")
                    nc.scalar.activation(
                        out=e_scr[:, 0:w], in_=sub[:, 0:w], func=AF.Exp,
                        bias=0.0, scale=INV_T, accum_out=sk[:],
                    )
                    if k == 0:
                        s_tot = sk
                    else:
                        s2 = stats_pool.tile([P, 1], F32, tag="s_tot",
                                             name=f"stot_{i}")
                        nc.vector.tensor_add(s2, s_tot, sk)
                        s_tot = s2
                nc.vector.tensor_copy(Sacc[:, i:i + 1], s_tot)

                # diagonal of G: wedge cols [128i, 128i+128)
                jd, doff = divmod(i * P, 512)
                dscr = dscr_pool.tile([P, P], F32, tag="dscr")
                nc.vector.tensor_mul(dscr[:], ps[jd][:, doff:doff + P], ident[:])
                dsum = stats_pool.tile([P, 1], F32, tag="dsum")
                nc.vector.reduce_sum(dsum, dscr[:], axis=AX.X)
                dm = stats_pool.tile([P, 1], F32, tag="dm")
                nc.vector.tensor_sub(dm, dsum, mm)
                nc.scalar.activation(
                    out=ecacc[:, i:i + 1], in_=dm[:], func=AF.Exp,
                    bias=0.0, scale=INV_T,
                )

                # wedge off-diagonal margin: mask the diagonal out of the
                # diag block's sub tile, then max over all wedge sub values.
                dmask = dscr_pool.tile([P, P], F32, tag="dmask")
                nc.vector.scalar_tensor_tensor(
                    out=dmask[:], in0=ident[:], scalar=NEG_BIG,
                    in1=subs[jd][:, doff:doff + P],
                    op0=AL.mult, op1=AL.add,
                )
                om = stats_pool.tile([P, 1], F32, tag="om")
                nc.vector.reduce_max(om, dmask[:], axis=AX.X)
                for k, (c0, c1) in enumerate(spans):
                    w = c1 - c0
                    segs = []
                    if k == jd:
                        lo_w, hi_0 = doff, doff + P
                        if lo_w > 0:
                            segs.append((0, lo_w))
                        if hi_0 < w:
                            segs.append((hi_0, w))
                    else:
                        segs.append((0, w))
                    for (s0, s1) in segs:
                        ok = stats_pool.tile([P, 1], F32, tag="ok",
                                             name=f"ok_{i}_{k}_{s0}")
                        nc.vector.reduce_max(ok, subs[k][:, s0:s1], axis=AX.X)
                        om2 = stats_pool.tile([P, 1], F32, tag="om",
                                              name=f"om_{i}_{k}_{s0}")
                        nc.vector.tensor_max(om2, om, ok)
                        om = om2
                # margin = -max(sub off-diag) = m - max_offdiag  (raw units)
                nc.vector.tensor_scalar_mul(offacc[:, i:i + 1], om, -1.0)

            # 5. final pass: loss = -Ln(ec/S + eps), one ACT table switch
            rS = singles.tile([P, NB], F32)
            nc.vector.reciprocal(rS, Sacc[:])
            r = singles.tile([P, NB], F32)
            nc.vector.tensor_mul(r, ecacc[:], rS)
            v = singles.tile([P, NB], F32)
            nc.vector.tensor_scalar_add(v, r, EPS)
            lnv = singles.tile([P, NB], F32)
            nc.scalar.activation(out=lnv[:], in_=v[:], func=AF.Ln,
                                 bias=0.0, scale=1.0)
            lossacc = singles.tile([P, NB], F32)
            nc.vector.tensor_scalar_mul(lossacc[:], lnv, -1.0)

            nc.sync.dma_start(out=loss[:], in_=lossacc[:])
            nc.sync.dma_start(out=mrow[:], in_=macc[:])
            nc.sync.dma_start(out=margin[:], in_=offacc[:])

    nc.finalize()
    return nc


_CACHED = {}


def _get_nc(debug=False):
    key = bool(debug)
    if key not in _CACHED:
        _CACHED[key] = _build(debug=debug)
    return _CACHED[key]


def _full_reference_fallback(xf):
    """Dense f64 computation, used only if the margin guarantee fails
    (never for the spec's N(0,1) inputs)."""
    losses = []
    for b in range(xf.shape[0]):
        g = (xf[b].astype(np.float64) @ xf[b].astype(np.float64).T) / 0.2
        m = g.max(axis=-1, keepdims=True)
        e = np.exp(g - m)
        diag = np.diagonal(e, axis1=-2, axis2=-1) / e.sum(axis=-1)
        losses.append(-np.log(diag.astype(np.float32) + np.float32(EPS)))
    return np.float32(np.mean(np.stack(losses)))


def kernel(x, _debug=False):
    x = np.ascontiguousarray(np.asarray(x, dtype=np.float32)).reshape(B, C, HW)
    nc = _get_nc(debug=_debug)
    in_maps = [{"x": x[b]} for b in range(B)]
    res = run_bass_kernel_spmd(nc, in_maps, core_ids=list(range(B)))

    rows = np.stack([res.results[b]["loss"] for b in range(B)])    # [B,128,NB]
    ms = np.stack([res.results[b]["mrow"] for b in range(B)])
    margins = np.stack([res.results[b]["margin"] for b in range(B)])

    # Validate the wedge shortcut (see module docstring).
    spread = float(ms.max() - ms.min())
    min_margin = float(margins.min())
    if not (min_margin > MARGIN_SLACK + spread):
        return np.asarray(_full_reference_fallback(x), dtype=np.float32)

    per_row = np.transpose(rows, (0, 2, 1)).reshape(B, C)
    out = np.float32(np.mean(per_row.astype(np.float32)))
    kernel._last = res
    kernel._per_row = per_row
    kernel._stats = (spread, min_margin)
    return np.asarray(out, dtype=np.float32)


# revision 21
# speedup vs baseline: 1.1467x; 1.1156x over previous
"""Decorrelation loss kernel for Trainium2 (Bass/Tile), 8-core SPMD.

Problem: x [8, 1024, 64, 64] f32.
  xf = x.reshape(B, C, HW)
  dot = xf @ xf^T / 0.2            per batch  -> [C, C]
  loss = mean(-log(diag(softmax(dot, -1)) + 1e-10))

Sharding: data-parallel over batch B across the 8 cores; each core handles
one batch element and emits per-row losses; the host averages.

Per-core pipeline:
  1. SWDGE cast-DMA loads x_b [1024, 4096] f32 from HBM into SBUF as bf16,
     one 128-row chunk at a time (cast happens inside the DMA datapath).
  2. HWDGE xbar DMA-transpose builds xT in SBUF as four column-quarter
     tiles [128p, 32q, 256c] (n = q*128 + p), so matmuls dep-wait only on
     the quarters they actually read.
  3. TensorE accumulates G row-strips in PSUM (bf16 inputs, f32 accum).
     Only the lower-triangle wedge of G (strip i: cols [0, 128*(i+1))) is
     computed: for N(0,1) inputs the Gram diagonal ||x_c||^2 (~4096 raw)
     exceeds off-diagonals (~+-700 raw) by a margin >> 21, so every softmax
     term of the skipped upper triangle underflows to +0.0 in f32 and the
     diagonal is always the row max -- the wedge row-stats are bit-identical
     to the full-row stats.  This is validated per run: the kernel also
     outputs per-row max and wedge off-diagonal margin, and the host falls
     back to a dense computation if the margin guarantee ever fails.
  4. Row stats on DVE/ACT: row max m (wedge), g-m (exact 0 at the max),
     ACT exp with accum_out -> row sum S; diagonal via identity mask.
  5. Final -log(diag_softmax + 1e-10) on the ACT Ln LUT (bit-matches the
     jax-on-neuron reference), deferred to a single pass at the end so the
     ACT table is not reloaded per strip.
"""

import numpy as np

import concourse.bass as bass
import concourse.mybir as mybir
from concourse import bacc
from concourse.tile import TileContext
from concourse.bass_utils import run_bass_kernel_spmd
from concourse.masks import make_identity

B, C, HW = 8, 1024, 4096
P = 128                     # partitions
NB = C // P                 # 8 row strips
KQ = HW // P                # 32 contraction sub-blocks
NQ = 4                      # xT column-quarter tiles
QW = C // NQ                # 256 columns per quarter
INV_T = 5.0                 # 1 / TEMPERATURE
EPS = 1e-10
NEG_BIG = -1.0e30

F32 = mybir.dt.float32
BF16 = mybir.dt.bfloat16
AF = mybir.ActivationFunctionType
AL = mybir.AluOpType
AX = mybir.AxisListType

# Host-side validity condition for the wedge shortcut (see module docstring):
# every skipped softmax term has argument 5*(g_cd - m_d), bounded above by
# 5*(spread - min_margin); f32 exp underflows to +0.0 below ~-104.  Require
# min_margin - spread > 30 raw units (5*30 = 150 >> 104, with slack).
MARGIN_SLACK = 30.0


def _build(debug=False):
    nc = bacc.Bacc(None, target_bir_lowering=False)

    x = nc.dram_tensor("x", [C, HW], F32, kind="ExternalInput")
    # packed output: cols [0:8) per-row loss, [8:16) row max, [16:24) margin
    out = nc.dram_tensor("out", [P, 3 * NB], F32, kind="ExternalOutput")

    with TileContext(nc) as tc:
        with (
            tc.tile_pool(name="singles", bufs=1) as singles,
            tc.tile_pool(name="xchunk", bufs=4) as xchunk_pool,
            tc.tile_pool(name="xt", bufs=1) as xt_pool,
            tc.tile_pool(name="psum", bufs=8, space="PSUM") as psum_pool,
            tc.tile_pool(name="escr", bufs=3) as escr_pool,
            tc.tile_pool(name="dscr", bufs=2) as dscr_pool,
            tc.tile_pool(name="stats", bufs=16) as stats_pool,
        ):
            ident = singles.tile([P, P], F32)
            make_identity(nc, ident)

            # NOTE: do NOT force a combined Exp+Ln ACT table set here: the
            # "natural_log_exp_and_others" set's Ln LUT differs from the
            # default "natural_log" set that the jax-on-neuron reference
            # uses (Ln(1.0) = 9.34e-10 vs 6.11e-13), which breaks the
            # bit-exact match.  The default per-func table loads match.

            # per-strip stashes, consumed by the final pass
            Sacc = singles.tile([P, NB], F32)
            ecacc = singles.tile([P, NB], F32)
            outacc = singles.tile([P, 3 * NB], F32)

            # xT quarters: [128p, 32q, 256c] bf16, n = q*128 + p
            xt_q = [
                xt_pool.tile([P, KQ, QW], BF16, tag=f"xt{t}", name=f"xt_{t}")
                for t in range(NQ)
            ]

            def xt_at(c0):
                """(quarter tile, local offset) containing column c0."""
                return xt_q[c0 // QW], c0 % QW

            # 1+2. cast-load + transpose, HIGHEST chunks first (strips are
            # upper-triangle wedges, so quarter 3 carries the most matmul
            # work and quarter 0 the least).  Pair-sized cast-loads with an
            # explicit transposes-before-next-load ordering dep give an
            # evenly spaced quarter delivery (~18us apart in the cost
            # model) that the PE stream below never stalls on.
            from concourse.tile import add_dep_helper
            prev_t = None
            for h in range(NB // 2 - 1, -1, -1):
                cb0 = 2 * h
                xchunk = xchunk_pool.tile([P, 2, HW], BF16, tag="xchunk",
                                          name=f"xchunk_{h}")
                ld = nc.gpsimd.dma_start(
                    out=xchunk[:],
                    in_=x[cb0 * P:(cb0 + 2) * P, :].rearrange(
                        "(cb p) n -> p cb n", p=P),
                )
                if prev_t is not None:
                    for t in prev_t:
                        add_dep_helper(ld.ins, t.ins, sync=True,
                                       reason="load after prev pair transposes")
                prev_t = []
                for cb in (cb0 + 1, cb0):
                    dst, off = xt_at(cb * P)
                    tr = nc.sync.dma_start_transpose(
                        out=dst[:, :, off:off + P],
                        in_=xchunk[:, cb - cb0, :],
                    )
                    prev_t.append(tr)

            # 3. matmuls: upper wedge per strip (cols [128i, 1024)), emitted
            # quarter-major (q3 first, matching reversed chunk delivery) so
            # the in-order PE stream never head-of-line blocks on a not-yet
            # -transposed quarter.  Two waves keep PSUM <= 8 banks:
            # wave A = strips 7..2 (1,1,1,1,2,2 banks), wave B = strips 1,0.
            strips = {}
            for i in range(NB):
                c_lo = P * i
                pieces = []
                c0 = c_lo
                while c0 < C:
                    c1 = min((c0 // QW + 1) * QW, C)
                    pieces.append((c0, c1))
                    c0 = c1
                strips[i] = {
                    "c_lo": c_lo,
                    "pieces": list(reversed(pieces)),
                    "banks": ([(c_lo, 512), (512, C)] if c_lo < 512
                              else [(c_lo, C)]),
                }

            def emit_strip_stats(i):
                st = strips[i]
                ps, banks = st["ps"], st["banks"]
                widths = [b1 - b0 for (b0, b1) in banks]
                # row max over the wedge (raw g, unscaled)
                mm = None
                for b, (pst, w) in enumerate(zip(ps, widths)):
                    mk = stats_pool.tile([P, 1], F32, tag=f"m{b}",
                                         name=f"m_{i}_{b}")
                    nc.vector.reduce_max(mk, pst[:, 0:w], axis=AX.X)
                    if mm is None:
                        mm = mk
                    else:
                        m2 = stats_pool.tile([P, 1], F32, tag="mm",
                                             name=f"mm_{i}_{b}")
                        nc.vector.tensor_max(m2, mm, mk)
                        mm = m2
                nc.vector.tensor_copy(outacc[:, NB + i:NB + i + 1], mm)

                # exp(5*(g - m)) with row-sum accumulation; subtract is a
                # separate DVE op so the max element cancels to exact 0.0
                subs = []
                s_tot = None
                for b, (pst, w) in enumerate(zip(ps, widths)):
                    sub = escr_pool.tile([P, 512], F32, tag="sub",
                                         name=f"sub_{i}_{b}")
                    nc.vector.tensor_scalar(
                        out=sub[:, 0:w], in0=pst[:, 0:w],
                        scalar1=mm[:], scalar2=None, op0=AL.subtract,
                    )
                    subs.append(sub)
                    e_scr = escr_pool.tile([P, 512], F32, tag="e",
                                           name=f"e_{i}_{b}")
                    sk = stats_pool.tile([P, 1], F32, tag=f"s{b}",
                                         name=f"s_{i}_{b}")
                    nc.scalar.activation(
                        out=e_scr[:, 0:w], in_=sub[:, 0:w], func=AF.Exp,
                        bias=0.0, scale=INV_T, accum_out=sk[:],
                    )
                    if s_tot is None:
                        s_tot = sk
                    else:
                        s2 = stats_pool.tile([P, 1], F32, tag="s_tot",
                                             name=f"stot_{i}_{b}")
                        nc.vector.tensor_add(s2, s_tot, sk)
                        s_tot = s2
                nc.vector.tensor_copy(Sacc[:, i:i + 1], s_tot)

                # diagonal of G: the first 128 wedge cols (bank 0 offset 0)
                jd, doff = 0, 0
                dscr = dscr_pool.tile([P, P], F32, tag="dscr")
                nc.vector.tensor_mul(dscr[:], ps[jd][:, doff:doff + P], ident[:])
                dsum = stats_pool.tile([P, 1], F32, tag="dsum")
                nc.vector.reduce_sum(dsum, dscr[:], axis=AX.X)
                dm = stats_pool.tile([P, 1], F32, tag="dm")
                nc.vector.tensor_sub(dm, dsum, mm)
                nc.scalar.activation(
                    out=ecacc[:, i:i + 1], in_=dm[:], func=AF.Exp,
                    bias=0.0, scale=INV_T,
                )

                # wedge off-diagonal margin: mask the diagonal out of the
                # diag block's sub tile, then max over all wedge sub values.
                dmask = dscr_pool.tile([P, P], F32, tag="dmask")
                nc.vector.scalar_tensor_tensor(
                    out=dmask[:], in0=ident[:], scalar=NEG_BIG,
                    in1=subs[jd][:, doff:doff + P],
                    op0=AL.mult, op1=AL.add,
                )
                om = stats_pool.tile([P, 1], F32, tag="om")
                nc.vector.reduce_max(om, dmask[:], axis=AX.X)
                for b, w in enumerate(widths):
                    segs = []
                    if b == jd:
                        if doff > 0:
                            segs.append((0, doff))
                        if doff + P < w:
                            segs.append((doff + P, w))
                    else:
                        segs.append((0, w))
                    for (s0, s1) in segs:
                        ok = stats_pool.tile([P, 1], F32, tag="ok",
                                             name=f"ok_{i}_{b}_{s0}")
                        nc.vector.reduce_max(ok, subs[b][:, s0:s1], axis=AX.X)
                        om2 = stats_pool.tile([P, 1], F32, tag="om",
                                              name=f"om_{i}_{b}_{s0}")
                        nc.vector.tensor_max(om2, om, ok)
                        om = om2
                # margin = -max(off-diag sub) = m - max_offdiag  (raw units)
                nc.vector.tensor_scalar_mul(
                    outacc[:, 2 * NB + i:2 * NB + i + 1], om, -1.0)

            # PE stream: quarter rounds globally ordered so the in-order
            # stream tracks the (reversed) chunk delivery.  Wave A =
            # strips 7..2 (8 PSUM banks), wave B = strips 1,0 (4 banks,
            # allocated after A's early strips release theirs).
            WAVE = {"A": [7, 6, 5, 4, 3, 2], "B": [1, 0]}
            ROUNDS = [("A", 3), ("A", 2), ("B", 3), ("B", 2),
                      ("A", 1), ("B", 1), ("B", 0)]
            for (w, qt) in ROUNDS:
                for i in WAVE[w]:
                    st = strips[i]
                    if "ps" not in st:
                        st["ps"] = [
                            psum_pool.tile([P, 512], F32, tag="ps",
                                           name=f"ps_{i}_{b}")
                            for b in range(len(st["banks"]))
                        ]
                    banks, ps = st["banks"], st["ps"]
                    lhs_t, lhs_off = xt_at(st["c_lo"])
                    for (c0, c1) in st["pieces"]:
                        if c0 // QW != qt:
                            continue
                        b = 0 if c0 < banks[0][1] else 1
                        p0 = c0 - banks[b][0]
                        rhs_t, ro = xt_at(c0)
                        for q in range(KQ):
                            nc.tensor.matmul(
                                ps[b][:, p0:p0 + (c1 - c0)],
                                lhsT=lhs_t[:, q, lhs_off:lhs_off + P],
                                rhs=rhs_t[:, q, ro:ro + (c1 - c0)],
                                start=(q == 0),
                                stop=(q == KQ - 1),
                            )
                    if qt == st["c_lo"] // QW:
                        # lowest quarter of this strip: wedge complete
                        emit_strip_stats(i)

            # 5. final pass: loss = -Ln(ec/S + eps), one ACT table switch
            rS = singles.tile([P, NB], F32)
            nc.vector.reciprocal(rS, Sacc[:])
            r = singles.tile([P, NB], F32)
            nc.vector.tensor_mul(r, ecacc[:], rS)
            v = singles.tile([P, NB], F32)
            nc.vector.tensor_scalar_add(v, r, EPS)
            lnv = singles.tile([P, NB], F32)
            nc.scalar.activation(out=lnv[:], in_=v[:], func=AF.Ln,
                                 bias=0.0, scale=1.0)
            nc.vector.tensor_scalar_mul(outacc[:, 0:NB], lnv[:], -1.0)

            nc.sync.dma_start(out=out[:], in_=outacc[:])

    nc.finalize()
    return nc


_CACHED = {}


def _get_nc(debug=False):
    key = bool(debug)
    if key not in _CACHED:
        _CACHED[key] = _build(debug=debug)
    return _CACHED[key]


def _full_reference_fallback(xf):
    """Dense f64 computation, used only if the margin guarantee fails
    (never for the spec's N(0,1) inputs)."""
    losses = []
    for b in range(xf.shape[0]):
        g = (xf[b].astype(np.float64) @ xf[b].astype(np.float64).T) / 0.2
        m = g.max(axis=-1, keepdims=True)
        e = np.exp(g - m)
        diag = np.diagonal(e, axis1=-2, axis2=-1) / e.sum(axis=-1)
        losses.append(-np.log(diag.astype(np.float32) + np.float32(EPS)))
    return np.float32(np.mean(np.stack(losses)))


def kernel(x, _debug=False):
    x = np.ascontiguousarray(np.asarray(x, dtype=np.float32)).reshape(B, C, HW)
    nc = _get_nc(debug=_debug)
    in_maps = [{"x": x[b]} for b in range(B)]
    res = run_bass_kernel_spmd(nc, in_maps, core_ids=list(range(B)))

    outs = np.stack([res.results[b]["out"] for b in range(B)])  # [B,128,3*NB]
    rows = outs[:, :, 0:NB]
    ms = outs[:, :, NB:2 * NB]
    margins = outs[:, :, 2 * NB:3 * NB]

    # Validate the wedge shortcut (see module docstring).
    spread = float(ms.max() - ms.min())
    min_margin = float(margins.min())
    if not (min_margin > MARGIN_SLACK + spread):
        return np.asarray(_full_reference_fallback(x), dtype=np.float32)

    per_row = np.transpose(rows, (0, 2, 1)).reshape(B, C)
    out = np.float32(np.mean(per_row.astype(np.float32)))
    kernel._last = res
    kernel._per_row = per_row
    kernel._stats = (spread, min_margin)
    return np.asarray(out, dtype=np.float32)


# revision 22
# speedup vs baseline: 21012.6551x; 18324.9763x over previous
"""Decorrelation loss kernel for Trainium2 (Bass/Tile), 8-core SPMD.

Problem: x [8, 1024, 64, 64] f32.
  xf = x.reshape(B, C, HW)
  dot = xf @ xf^T / 0.2            per batch  -> [C, C]
  loss = mean(-log(diag(softmax(dot, -1)) + 1e-10))

Sharding: data-parallel over batch B across the 8 cores; each core handles
one batch element and emits per-row losses; the host averages.

Per-core pipeline:
  1. SWDGE cast-DMA loads x_b [1024, 4096] f32 from HBM into SBUF as bf16,
     one 128-row chunk at a time (cast happens inside the DMA datapath).
  2. HWDGE xbar DMA-transpose builds xT in SBUF as four column-quarter
     tiles [128p, 32q, 256c] (n = q*128 + p), so matmuls dep-wait only on
     the quarters they actually read.
  3. TensorE accumulates G row-strips in PSUM (bf16 inputs, f32 accum).
     Only the upper-triangle wedge of G (strip i: cols [128i, 1024)) is
     computed: for N(0,1) inputs the Gram diagonal ||x_c||^2 (~4096 raw)
     exceeds off-diagonals (~+-700 raw) by a margin >> 21, so every softmax
     term of the skipped lower triangle underflows to +0.0 in f32 and the
     diagonal is always the row max -- the wedge row-stats are bit-identical
     to the full-row stats.  This is validated per run: the kernel also
     outputs per-row max and wedge off-diagonal margin, and the host falls
     back to a dense computation if the margin guarantee ever fails.
     Strips are emitted 7->0, quarter-major, matching the reversed chunk
     delivery so the in-order PE stream rarely stalls.
  4. Row stats on DVE/ACT: row max m (wedge), g-m (exact 0 at the max),
     ACT exp with accum_out -> row sum S; diagonal via identity mask.
  5. Final -log(diag_softmax + 1e-10) on the ACT Ln LUT (bit-matches the
     jax-on-neuron reference), deferred to a single pass at the end so the
     ACT table is not reloaded per strip.
"""

import numpy as np

import concourse.bass as bass
import concourse.mybir as mybir
from concourse import bacc
from concourse.tile import TileContext
from concourse.bass_utils import run_bass_kernel_spmd
from concourse.masks import make_identity

B, C, HW = 8, 1024, 4096
P = 128                     # partitions
NB = C // P                 # 8 row strips
KQ = HW // P                # 32 contraction sub-blocks
NQ = 4                      # xT column-quarter tiles
QW = C // NQ                # 256 columns per quarter
INV_T = 5.0                 # 1 / TEMPERATURE
EPS = 1e-10
NEG_BIG = -1.0e30

F32 = mybir.dt.float32
BF16 = mybir.dt.bfloat16
AF = mybir.ActivationFunctionType
AL = mybir.AluOpType
AX = mybir.AxisListType

# Host-side validity condition for the wedge shortcut (see module docstring):
# every skipped softmax term has argument 5*(g_cd - m_d), bounded above by
# 5*(spread - min_margin); f32 exp underflows to +0.0 below ~-104.  Require
# min_margin - spread > 30 raw units (5*30 = 150 >> 104, with slack).
MARGIN_SLACK = 30.0


def _build(debug=False):
    nc = bacc.Bacc(None, target_bir_lowering=False)

    x = nc.dram_tensor("x", [C, HW], F32, kind="ExternalInput")
    # packed output: cols [0:8) per-row loss, [8:16) row max, [16:24) margin
    out = nc.dram_tensor("out", [P, 3 * NB], F32, kind="ExternalOutput")

    with TileContext(nc) as tc:
        with (
            tc.tile_pool(name="singles", bufs=1) as singles,
            tc.tile_pool(name="xchunk", bufs=4) as xchunk_pool,
            tc.tile_pool(name="xt", bufs=1) as xt_pool,
            tc.tile_pool(name="psum", bufs=8, space="PSUM") as psum_pool,
            tc.tile_pool(name="escr", bufs=3) as escr_pool,
            tc.tile_pool(name="dscr", bufs=2) as dscr_pool,
            tc.tile_pool(name="stats", bufs=16) as stats_pool,
        ):
            ident = singles.tile([P, P], F32)
            make_identity(nc, ident)

            # NOTE: do NOT force a combined Exp+Ln ACT table set here: the
            # "natural_log_exp_and_others" set's Ln LUT differs from the
            # default "natural_log" set that the jax-on-neuron reference
            # uses (Ln(1.0) = 9.34e-10 vs 6.11e-13), which breaks the
            # bit-exact match.  The default per-func table loads match.

            # per-strip stashes, consumed by the final pass
            Sacc = singles.tile([P, NB], F32)
            ecacc = singles.tile([P, NB], F32)
            outacc = singles.tile([P, 3 * NB], F32)

            # xT quarters: [128p, 32q, 256c] bf16, n = q*128 + p
            xt_q = [
                xt_pool.tile([P, KQ, QW], BF16, tag=f"xt{t}", name=f"xt_{t}")
                for t in range(NQ)
            ]

            def xt_at(c0):
                """(quarter tile, local offset) containing column c0."""
                return xt_q[c0 // QW], c0 % QW

            # 1+2. cast-load + transpose, HIGHEST chunks first (strips are
            # upper-triangle wedges, so quarter 3 carries the most matmul
            # work and quarter 0 the least).  Pair-sized cast-loads with an
            # explicit transposes-before-next-load ordering dep give an
            # evenly spaced quarter delivery (~18us apart in the cost
            # model) that the PE stream below never stalls on.
            from concourse.tile import add_dep_helper
            prev_t = None
            for h in range(NB // 2 - 1, -1, -1):
                cb0 = 2 * h
                xchunk = xchunk_pool.tile([P, 2, HW], BF16, tag="xchunk",
                                          name=f"xchunk_{h}")
                ld = nc.gpsimd.dma_start(
                    out=xchunk[:],
                    in_=x[cb0 * P:(cb0 + 2) * P, :].rearrange(
                        "(cb p) n -> p cb n", p=P),
                )
                if prev_t is not None:
                    for t in prev_t:
                        add_dep_helper(ld.ins, t.ins, sync=True,
                                       reason="load after prev pair transposes")
                prev_t = []
                for cb in (cb0 + 1, cb0):
                    dst, off = xt_at(cb * P)
                    tr = nc.sync.dma_start_transpose(
                        out=dst[:, :, off:off + P],
                        in_=xchunk[:, cb - cb0, :],
                    )
                    prev_t.append(tr)

            # 3. matmuls: upper wedge per strip (cols [128i, 1024)), emitted
            # quarter-major (q3 first, matching reversed chunk delivery) so
            # the in-order PE stream never head-of-line blocks on a not-yet
            # -transposed quarter.  Two waves keep PSUM <= 8 banks:
            # wave A = strips 7..2 (1,1,1,1,2,2 banks), wave B = strips 1,0.
            strips = {}
            for i in range(NB):
                c_lo = P * i
                pieces = []
                c0 = c_lo
                while c0 < C:
                    c1 = min((c0 // QW + 1) * QW, C)
                    pieces.append((c0, c1))
                    c0 = c1
                strips[i] = {
                    "c_lo": c_lo,
                    "pieces": list(reversed(pieces)),
                    "banks": ([(c_lo, 512), (512, C)] if c_lo < 512
                              else [(c_lo, C)]),
                }

            def emit_strip_stats(i):
                st = strips[i]
                ps, banks = st["ps"], st["banks"]
                widths = [b1 - b0 for (b0, b1) in banks]
                # row max over the wedge (raw g, unscaled)
                mm = None
                for b, (pst, w) in enumerate(zip(ps, widths)):
                    mk = stats_pool.tile([P, 1], F32, tag=f"m{b}",
                                         name=f"m_{i}_{b}")
                    nc.vector.reduce_max(mk, pst[:, 0:w], axis=AX.X)
                    if mm is None:
                        mm = mk
                    else:
                        m2 = stats_pool.tile([P, 1], F32, tag="mm",
                                             name=f"mm_{i}_{b}")
                        nc.vector.tensor_max(m2, mm, mk)
                        mm = m2
                nc.vector.tensor_copy(outacc[:, NB + i:NB + i + 1], mm)

                # exp(5*(g - m)) with row-sum accumulation; subtract is a
                # separate DVE op so the max element cancels to exact 0.0
                subs = []
                s_tot = None
                for b, (pst, w) in enumerate(zip(ps, widths)):
                    sub = escr_pool.tile([P, 512], F32, tag="sub",
                                         name=f"sub_{i}_{b}")
                    nc.vector.tensor_scalar(
                        out=sub[:, 0:w], in0=pst[:, 0:w],
                        scalar1=mm[:], scalar2=None, op0=AL.subtract,
                    )
                    subs.append(sub)
                    e_scr = escr_pool.tile([P, 512], F32, tag="e",
                                           name=f"e_{i}_{b}")
                    sk = stats_pool.tile([P, 1], F32, tag=f"s{b}",
                                         name=f"s_{i}_{b}")
                    nc.scalar.activation(
                        out=e_scr[:, 0:w], in_=sub[:, 0:w], func=AF.Exp,
                        bias=0.0, scale=INV_T, accum_out=sk[:],
                    )
                    if s_tot is None:
                        s_tot = sk
                    else:
                        s2 = stats_pool.tile([P, 1], F32, tag="s_tot",
                                             name=f"stot_{i}_{b}")
                        nc.vector.tensor_add(s2, s_tot, sk)
                        s_tot = s2
                nc.vector.tensor_copy(Sacc[:, i:i + 1], s_tot)

                # diagonal of G: the first 128 wedge cols (bank 0 offset 0)
                jd, doff = 0, 0
                dscr = dscr_pool.tile([P, P], F32, tag="dscr")
                nc.vector.tensor_mul(dscr[:], ps[jd][:, doff:doff + P], ident[:])
                dsum = stats_pool.tile([P, 1], F32, tag="dsum")
                nc.vector.reduce_sum(dsum, dscr[:], axis=AX.X)
                dm = stats_pool.tile([P, 1], F32, tag="dm")
                nc.vector.tensor_sub(dm, dsum, mm)
                nc.scalar.activation(
                    out=ecacc[:, i:i + 1], in_=dm[:], func=AF.Exp,
                    bias=0.0, scale=INV_T,
                )

                # wedge off-diagonal margin: mask the diagonal out of the
                # diag block's sub tile, then max over all wedge sub values.
                dmask = dscr_pool.tile([P, P], F32, tag="dmask")
                nc.vector.scalar_tensor_tensor(
                    out=dmask[:], in0=ident[:], scalar=NEG_BIG,
                    in1=subs[jd][:, doff:doff + P],
                    op0=AL.mult, op1=AL.add,
                )
                om = stats_pool.tile([P, 1], F32, tag="om")
                nc.vector.reduce_max(om, dmask[:], axis=AX.X)
                for b, w in enumerate(widths):
                    segs = []
                    if b == jd:
                        if doff > 0:
                            segs.append((0, doff))
                        if doff + P < w:
                            segs.append((doff + P, w))
                    else:
                        segs.append((0, w))
                    for (s0, s1) in segs:
                        ok = stats_pool.tile([P, 1], F32, tag="ok",
                                             name=f"ok_{i}_{b}_{s0}")
                        nc.vector.reduce_max(ok, subs[b][:, s0:s1], axis=AX.X)
                        om2 = stats_pool.tile([P, 1], F32, tag="om",
                                              name=f"om_{i}_{b}_{s0}")
                        nc.vector.tensor_max(om2, om, ok)
                        om = om2
                # margin = -max(off-diag sub) = m - max_offdiag  (raw units)
                nc.vector.tensor_scalar_mul(
                    outacc[:, 2 * NB + i:2 * NB + i + 1], om, -1.0)

            # PE stream: quarter rounds globally ordered so the in-order
            # stream tracks the (reversed) chunk delivery.  Wave A =
            # strips 7..2 (8 PSUM banks), wave B = strips 1,0 (4 banks,
            # allocated after A's early strips release theirs).
            WAVE = {"A": [7, 6, 5, 4, 3, 2], "B": [1, 0]}
            ROUNDS = [("A", 3), ("A", 2), ("B", 3), ("B", 2),
                      ("A", 1), ("B", 1), ("B", 0)]
            for (w, qt) in ROUNDS:
                for i in WAVE[w]:
                    st = strips[i]
                    if "ps" not in st:
                        st["ps"] = [
                            psum_pool.tile([P, 512], F32, tag="ps",
                                           name=f"ps_{i}_{b}")
                            for b in range(len(st["banks"]))
                        ]
                    banks, ps = st["banks"], st["ps"]
                    lhs_t, lhs_off = xt_at(st["c_lo"])
                    for (c0, c1) in st["pieces"]:
                        if c0 // QW != qt:
                            continue
                        b = 0 if c0 < banks[0][1] else 1
                        p0 = c0 - banks[b][0]
                        rhs_t, ro = xt_at(c0)
                        for q in range(KQ):
                            nc.tensor.matmul(
                                ps[b][:, p0:p0 + (c1 - c0)],
                                lhsT=lhs_t[:, q, lhs_off:lhs_off + P],
                                rhs=rhs_t[:, q, ro:ro + (c1 - c0)],
                                start=(q == 0),
                                stop=(q == KQ - 1),
                            )
                    if qt == st["c_lo"] // QW:
                        # lowest quarter of this strip: wedge complete
                        emit_strip_stats(i)

            # 5. final pass: loss = -Ln(ec/S + eps), one ACT table switch
            rS = singles.tile([P, NB], F32)
            nc.vector.reciprocal(rS, Sacc[:])
            r = singles.tile([P, NB], F32)
            nc.vector.tensor_mul(r, ecacc[:], rS)
            v = singles.tile([P, NB], F32)
            nc.vector.tensor_scalar_add(v, r, EPS)
            lnv = singles.tile([P, NB], F32)
            nc.scalar.activation(out=lnv[:], in_=v[:], func=AF.Ln,
                                 bias=0.0, scale=1.0)
            nc.vector.tensor_scalar_mul(outacc[:, 0:NB], lnv[:], -1.0)

            nc.sync.dma_start(out=out[:], in_=outacc[:])

    nc.finalize()
    return nc


_CACHED = {}


def _get_nc(debug=False):
    key = bool(debug)
    if key not in _CACHED:
        _CACHED[key] = _build(debug=debug)
    return _CACHED[key]


def _full_reference_fallback(xf):
    """Dense f64 computation, used only if the margin guarantee fails
    (never for the spec's N(0,1) inputs)."""
    losses = []
    for b in range(xf.shape[0]):
        g = (xf[b].astype(np.float64) @ xf[b].astype(np.float64).T) / 0.2
        m = g.max(axis=-1, keepdims=True)
        e = np.exp(g - m)
        diag = np.diagonal(e, axis1=-2, axis2=-1) / e.sum(axis=-1)
        losses.append(-np.log(diag.astype(np.float32) + np.float32(EPS)))
    return np.float32(np.mean(np.stack(losses)))


def kernel(x, _debug=False):
    x = np.ascontiguousarray(np.asarray(x, dtype=np.float32)).reshape(B, C, HW)
    nc = _get_nc(debug=_debug)
    in_maps = [{"x": x[b]} for b in range(B)]
    res = run_bass_kernel_spmd(nc, in_maps, core_ids=list(range(B)))

    outs = np.stack([res.results[b]["out"] for b in range(B)])  # [B,128,3*NB]
    rows = outs[:, :, 0:NB]
    ms = outs[:, :, NB:2 * NB]
    margins = outs[:, :, 2 * NB:3 * NB]

    # Validate the wedge shortcut (see module docstring).
    spread = float(ms.max() - ms.min())
    min_margin = float(margins.min())
    if not (min_margin > MARGIN_SLACK + spread):
        return np.asarray(_full_reference_fallback(x), dtype=np.float32)

    per_row = np.transpose(rows, (0, 2, 1)).reshape(B, C)
    out = np.float32(np.mean(per_row.astype(np.float32)))
    kernel._last = res
    kernel._per_row = per_row
    kernel._stats = (spread, min_margin)
    return np.asarray(out, dtype=np.float32)
